# revision 1
# baseline (speedup 1.0000x reference)
"""DGCNN segmentation forward pass on 8 Trainium2 NeuronCores.

Data-parallel over batch: core c processes cloud c. Training-mode BatchNorm
statistics couple the batch, so each BN layer does a tiny cross-core
AllReduce of per-channel (sum, sumsq) between compute phases.

Self-contained: hardcodes B=8, C=6, N=4096, K=20.
"""
import numpy as np

import concourse.bacc as bacc
import concourse.bass as bass
import concourse.mybir as mybir
from concourse.tile import TileContext
from concourse.masks import make_identity

P = 128
K = 20
EPS = 1e-5
BIG = float(3 * 2 ** 34)   # 1.5*2^35: signed round-to-4096-grid magic bias
CREC = float(3 * 2 ** 22)  # 1.5*2^23: signed round-to-int magic (index recovery)
F32 = mybir.dt.float32
F32R = mybir.dt.float32r
U32 = mybir.dt.uint32
AF = mybir.ActivationFunctionType
ALU = mybir.AluOpType
AX = mybir.AxisListType


USE_F32R_DIST = False     # distance matmuls (feeds knn selection)
USE_F32R_CONV = False     # conv / head matmuls


def r32(ap):
    return ap.bitcast(F32R) if USE_F32R_CONV else ap


def r32d(ap):
    return ap.bitcast(F32R) if USE_F32R_DIST else ap


def build_graph(nc, ins, outs, N, n_cores, lrelu_native=True, dbg=None,
                skip_collectives=False):
    """Emit the whole forward pass. ins/outs: dicts of DRAM APs."""
    NT = N // P
    NC512 = N // 512
    CNT_E = float(n_cores * N * K)   # BN sample count, conv2d layers
    CNT_N = float(n_cores * N)       # BN sample count, conv1d layers
    RG = [list(range(n_cores))]

    x_in = ins["x"]                  # [6, N]
    out_dram = outs["out"]           # [6, N]

    # scratch DRAM
    at_d = [nc.dram_tensor(f"at{i}", [N, 64], F32, kind="Internal") for i in range(3)]
    iwd_d = [nc.dram_tensor(f"iwd{i}", [16, (N // P) * K * 8], mybir.dt.int16,
                            kind="Internal") for i in range(3)]
    stg_d = [nc.dram_tensor(f"stg{i}", [64, N * K], F32, kind="Internal") for i in range(2)]
    n_ar = 0

    with TileContext(nc) as tc:
        from contextlib import ExitStack
        es = ExitStack()
        lp = es.enter_context(tc.tile_pool(name="long", bufs=1))

        ident = lp.tile([P, P], F32)
        make_identity(nc, ident[:])
        ones_col = lp.tile([P, 1], F32)
        nc.vector.memset(ones_col, 1.0)

        x12 = lp.tile([P, N], F32)       # x1 (rows 0:64) and x2 (rows 64:128)
        x3t = lp.tile([64, N], F32)
        maxY = lp.tile([64, N], F32)     # per-EC max-over-k staging (also EC3)
        maxE3 = maxY
        x2t = lp.tile([64, N], F32)      # base-0 copy of x2 (engines cannot
                                         # write across partition bases)

        # ---------- helpers ----------
        def allreduce_stats(st_ap, rows, cols):
            """st_ap: SBUF [rows, cols] -> allreduced in place."""
            nonlocal n_ar
            i = n_ar
            n_ar += 1
            cci = nc.dram_tensor(f"cci{i}", [rows, cols], F32, kind="Internal")
            cco = nc.dram_tensor(f"cco{i}", [rows, cols], F32, kind="Internal",
                                 addr_space="Shared")
            nc.sync.dma_start(out=cci.ap(), in_=st_ap)
            if not skip_collectives:
                nc.gpsimd.collective_compute(
                    kind="AllReduce", op=ALU.add, replica_groups=RG,
                    ins=[cci.ap()], outs=[cco.ap()])
                nc.sync.dma_start(out=st_ap, in_=cco.ap())

        def bn_coeffs(pool, st, gb, cnt, rows, ncols=1):
            """st: [rows, 2*ncols] (sum, sumsq pairs), gb: [rows, 2*ncols]
            (g, b pairs). Returns scale/shift [rows, ncols] tiles."""
            mean = pool.tile([rows, ncols], F32, tag="bn_mean")
            ex2 = pool.tile([rows, ncols], F32, tag="bn_ex2")
            scale = pool.tile([rows, ncols], F32, tag="bn_scale")
            shift = pool.tile([rows, ncols], F32, tag="bn_shift")
            s_cols = st[:, 0:2 * ncols:2]
            q_cols = st[:, 1:2 * ncols:2]
            nc.vector.tensor_scalar_mul(mean, s_cols, 1.0 / cnt)
            nc.vector.tensor_scalar_mul(ex2, q_cols, 1.0 / cnt)
            msq = pool.tile([rows, ncols], F32, tag="bn_msq")
            nc.vector.tensor_tensor(msq, mean, mean, op=ALU.mult)
            nc.vector.tensor_sub(ex2, ex2, msq)          # var
            std = pool.tile([rows, ncols], F32, tag="bn_std")
            epst = pool.tile([rows, 1], F32, tag="bn_eps")
            nc.vector.memset(epst, EPS)
            nc.scalar.activation(std, ex2, AF.Sqrt, bias=epst[:])
            nc.vector.reciprocal(scale, std)             # 1/std
            nc.vector.tensor_tensor(scale, scale, gb[:, 0:2 * ncols:2], op=ALU.mult)
            nc.vector.tensor_tensor(msq, mean, scale, op=ALU.mult)
            nc.vector.tensor_sub(shift, gb[:, 1:2 * ncols:2], msq)
            return scale[:], shift[:]

        def emit_lrelu(pool, out_ap, in_ap, scale, bias, accum_ap=None, tag="lr"):
            """out = lrelu(in*scale + bias); optional accum_ap [P,1] = row sums
            of the OUTPUT."""
            if lrelu_native:
                kw = {"accum_out": accum_ap} if accum_ap is not None else {}
                nc.scalar.activation(out_ap, in_ap, AF.Lrelu, bias=bias, scale=scale,
                                     alpha=0.2, **kw)
            else:
                nc.scalar.activation(out_ap, in_ap, AF.Identity, bias=bias, scale=scale)
                t = pool.tile([out_ap.shape[0], out_ap.free_size()], F32, tag=tag)
                nc.vector.tensor_scalar(t[:], out_ap, 0.2, None, op0=ALU.mult)
                nc.vector.tensor_tensor(out_ap, out_ap, t[:], op=ALU.max)
                if accum_ap is not None:
                    nc.vector.tensor_reduce(out=accum_ap, in_=out_ap, op=ALU.add,
                                            axis=AX.X)

        def build_aug(pool, src_ap, cin, from_dram=False):
            """Return (augL [cin+1, N] = [2x;1], augR [cin+1, N] = [x;-|x|^2])."""
            augL = pool.tile([cin + 1, N], F32, tag="augL")
            augR = pool.tile([cin + 1, N], F32, tag="augR")
            if from_dram:
                nc.sync.dma_start(out=augR[:cin, :], in_=src_ap)
            else:
                nc.scalar.copy(augR[:cin, :], src_ap)
            nc.scalar.activation(augL[:cin, :], augR[:cin, :], AF.Copy, bias=0.0,
                                 scale=2.0)
            onesrow = pool.tile([1, 512], F32, tag="onesrow")
            nc.vector.memset(onesrow, 1.0)
            with tc.tile_pool(name="sqp", bufs=2, space="PSUM") as sqp, \
                 tc.tile_pool(name="sqs", bufs=2) as sqs:
                for ch in range(NC512):
                    sl = slice(ch * 512, (ch + 1) * 512)
                    xsq = sqs.tile([cin, 512], F32, tag="xsq")
                    nc.scalar.activation(xsq, augR[:cin, sl], AF.Square)
                    ps = sqp.tile([1, 512], F32, tag="sq_ps")
                    nc.tensor.matmul(ps, lhsT=ones_col[:cin, :], rhs=xsq[:],
                                     start=True, stop=True)
                    srow = sqs.tile([1, 512], F32, tag="srow")
                    nc.scalar.activation(srow, ps, AF.Copy, bias=0.0, scale=-1.0)
                    nc.sync.dma_start(out=augR[cin:cin + 1, sl], in_=srow[:])
                    nc.sync.dma_start(out=augL[cin:cin + 1, sl], in_=onesrow[:])
            if dbg is not None:
                nc.sync.dma_start(out=dbg[f"srow{cin}"], in_=augR[cin:cin + 1, :])
            return augL, augR

        def build_tables(pool, augR, cin, wdT, wcdT, at_dram, bvT):
            """AT table -> DRAM, BvT -> SBUF [P, NT*64]."""
            with tc.tile_pool(name="tbl", bufs=3, space="PSUM") as tp, \
                 tc.tile_pool(name="tbls", bufs=3) as tsb:
                for t in range(NT):
                    xc = augR[:cin, t * P:(t + 1) * P]
                    ps_a = tp.tile([P, 64], F32, tag="at_ps")
                    nc.tensor.matmul(ps_a, lhsT=xc, rhs=wdT, start=True, stop=True)
                    a_sb = tsb.tile([P, 64], F32, tag="at_sb")
                    nc.scalar.copy(a_sb, ps_a)
                    nc.gpsimd.dma_start(out=at_dram.ap()[t * P:(t + 1) * P, :], in_=a_sb)
                    ps_b = tp.tile([P, 64], F32, tag="bv_ps")
                    nc.tensor.matmul(ps_b, lhsT=xc, rhs=wcdT, start=True, stop=True)
                    nc.scalar.copy(bvT[:, t * 64:(t + 1) * 64], ps_b)

        def edgeconv_passA(ec, pool, augL, augR, cin, at_dram, stg, gb_a,
                           enc_s4096=8192.0):
            sumA = pool.tile([P, 64], F32, tag="sumA")
            sqA = pool.tile([P, 64], F32, tag="sqA")
            nc.vector.memset(sumA, 0.0)
            nc.vector.memset(sqA, 0.0)
            with tc.tile_pool(name=f"pa{ec}", bufs=2) as wp, \
                 tc.tile_pool(name=f"pap{ec}", bufs=2, space="PSUM") as pp:
                for t in range(NT):
                    dtile = wp.tile([P, N], F32, tag="dtile")
                    for ch in range(NC512):
                        dps = pp.tile([P, 512], F32, tag="dps")
                        nc.tensor.matmul(dps, lhsT=r32d(augL[:, t * P:(t + 1) * P]),
                                         rhs=r32d(augR[:, ch * 512:(ch + 1) * 512]),
                                         start=True, stop=True)
                        nc.scalar.copy(dtile[:, ch * 512:(ch + 1) * 512], dps)
                    vals = wp.tile([P, 24], F32, tag="vals")
                    idxs = wp.tile([P, 24], U32, tag="idxs")
                    CH = N // 32
                    cand = wp.tile([P, 256], F32, tag="cand")
                    for c_ in range(32):
                        nc.vector.max(out=cand[:, 8 * c_:8 * c_ + 8],
                                      in_=dtile[:, c_ * CH:(c_ + 1) * CH])
                    for r in range(3):
                        v8 = vals[:, 8 * r:8 * r + 8]
                        nc.vector.max(out=v8, in_=cand[:])
                        if r < 2:
                            nc.vector.match_replace(out=cand[:], in_to_replace=v8,
                                                    in_values=cand[:], imm_value=-1e30)
                    for r in range(3):
                        nc.vector.max_index(out=idxs[:, 8 * r:8 * r + 8],
                                            in_max=vals[:, 8 * r:8 * r + 8],
                                            in_values=dtile[:])
                    if dbg is not None and t == 0:
                        nc.sync.dma_start(out=dbg[f"idx{ec}"], in_=idxs[:])
                        nc.sync.dma_start(out=dbg[f"vals{ec}"], in_=vals[:])
                    ep = wp.tile([P, K * 64], F32, tag="ep")
                    for s_ in range(K):
                        nc.gpsimd.indirect_dma_start(
                            out=ep[:, s_ * 64:(s_ + 1) * 64], out_offset=None,
                            in_=at_dram.ap(),
                            in_offset=bass.IndirectOffsetOnAxis(
                                ap=idxs[:, s_:s_ + 1], axis=0))
                    if dbg is not None and t == 0:
                        nc.sync.dma_start(out=dbg[f"graw{ec}"], in_=ep[:])
                    for s_ in range(K):
                        nc.vector.tensor_add(
                            ep[:, s_ * 64:(s_ + 1) * 64],
                            ep[:, s_ * 64:(s_ + 1) * 64],
                            bvT[:, t * 64:(t + 1) * 64])
                    if dbg is not None and t == 0:
                        nc.sync.dma_start(out=dbg[f"ep{ec}"], in_=ep[:])
                    red = wp.tile([P, 64], F32, tag="red")
                    nc.vector.tensor_reduce(out=red, in_=ep[:].rearrange("p (k c) -> p c k", k=K),
                                            op=ALU.add, axis=AX.X)
                    nc.vector.tensor_add(sumA, sumA, red)
                    sq = wp.tile([P, K * 64], F32, tag="eT")
                    nc.scalar.activation(sq, ep[:], AF.Square)
                    nc.vector.tensor_reduce(out=red, in_=sq[:].rearrange("p (k c) -> p c k", k=K),
                                            op=ALU.add, axis=AX.X)
                    nc.vector.tensor_add(sqA, sqA, red)
                    if ec < 2:
                        eT = wp.tile([64, K * P], F32, tag="eT")
                        for blk in range(5):
                            tps = pp.tile([64, 512], F32, tag="tps")
                            for s4 in range(4):
                                s = blk * 4 + s4
                                nc.tensor.transpose(tps[:, s4 * P:(s4 + 1) * P],
                                                    ep[:, s * 64:(s + 1) * 64], ident[:])
                            nc.scalar.copy(eT[:, blk * 512:(blk + 1) * 512], tps)
                        nc.sync.dma_start(out=stg.ap()[:, t * (K * P):(t + 1) * (K * P)], in_=eT)
                    else:
                        m3 = wp.tile([P, 64], F32, tag="m3")
                        nc.vector.tensor_reduce(out=m3, in_=ep[:].rearrange("p (k c) -> p c k", k=K),
                                                op=ALU.max, axis=AX.X)
                        mps = pp.tile([64, P], F32, tag="mps")
                        nc.tensor.transpose(mps, m3[:], ident[:])
                        nc.scalar.copy(maxE3[:, t * P:(t + 1) * P], mps)
                stp = pp.tile([64, 2], F32, tag="stp")
                nc.tensor.matmul(stp[:, 0:1], lhsT=sumA[:], rhs=ones_col[:], start=True, stop=True)
                nc.tensor.matmul(stp[:, 1:2], lhsT=sqA[:], rhs=ones_col[:], start=True, stop=True)
                st = pool.tile([64, 2], F32, tag=f"st_a{ec}")
                nc.scalar.copy(st, stp)
            allreduce_stats(st[:], 64, 2)
            if dbg is not None:
                nc.sync.dma_start(out=dbg[f"sta{ec}"], in_=st[:])
            return bn_coeffs(pool, st[:], gb_a, CNT_E, 64)

        def edgeconv_passB(ec, pool, stg, wbT, sc_a, sh_a, gb_b, x_dst):
            """load staged e_preT, bn+lrelu, conv-b, max-over-k, stats; then
            allreduce + write x_dst = lrelu(bn_b(maxY))."""
            accS = pool.tile([64, NT], F32, tag="accS")      # sum of e1 per tile
            sqB = pool.tile([64, NT * 5], F32, tag="sqB")    # sumsq of y per chunk
            with tc.tile_pool(name=f"pb{ec}", bufs=2) as wp, \
                 tc.tile_pool(name=f"pbp{ec}", bufs=2, space="PSUM") as pp:
                for t in range(NT):
                    eT = wp.tile([64, K * P], F32, tag="eTb")
                    nc.sync.dma_start(out=eT, in_=stg.ap()[:, t * (K * P):(t + 1) * (K * P)])
                    e1 = eT   # lrelu applied in place to save SBUF
                    emit_lrelu(wp, e1[:], eT[:], sc_a, sh_a, accum_ap=accS[:, t:t + 1],
                               tag="lrB")
                    scr = wp.tile([64, 512], F32, tag="scr")
                    for ch in range(5):
                        cps = pp.tile([64, 512], F32, tag="cps")
                        nc.tensor.matmul(cps, lhsT=r32(wbT), rhs=r32(e1[:, ch * 512:(ch + 1) * 512]),
                                         start=True, stop=True)
                        nc.scalar.activation(scr, cps, AF.Square,
                                             accum_out=sqB[:, t * 5 + ch:t * 5 + ch + 1])
                        mx = wp.tile([64, P], F32, tag="mx")
                        nc.vector.tensor_reduce(
                            out=mx, in_=cps[:].rearrange("c (s p) -> c p s", s=4),
                            op=ALU.max, axis=AX.X)
                        if ch == 0:
                            nc.vector.tensor_copy(maxY[:, t * P:(t + 1) * P], mx)
                        else:
                            nc.vector.tensor_tensor(maxY[:, t * P:(t + 1) * P],
                                                    maxY[:, t * P:(t + 1) * P], mx, op=ALU.max)
                # stats: sum_y = W_b @ sum_e1 ; sumsq from sqB
                st = pool.tile([64, 2], F32, tag=f"st_b{ec}")
                se = pool.tile([64, 1], F32, tag="se")
                nc.vector.tensor_reduce(out=se, in_=accS[:], op=ALU.add, axis=AX.X)
                sp = pp.tile([64, 1], F32, tag="sp")
                nc.tensor.matmul(sp, lhsT=wbT, rhs=se[:], start=True, stop=True)
                nc.scalar.copy(st[:, 0:1], sp)
                nc.vector.tensor_reduce(out=st[:, 1:2], in_=sqB[:], op=ALU.add, axis=AX.X)
            allreduce_stats(st[:], 64, 2)
            sc_b, sh_b = bn_coeffs(pool, st[:], gb_b, CNT_E, 64)
            emit_lrelu(pool, x_dst, maxY[:], sc_b, sh_b, tag="lrX")

        # ================= EC1 =================
        with tc.tile_pool(name="ec", bufs=1) as ecp:
            bvT = ecp.tile([P, NT * 64], F32, tag="bvT")
            augL, augR = build_aug(ecp, x_in, 6, from_dram=True)
            wd1 = ecp.tile([6, 64], F32, tag="wd1")
            wcd1 = ecp.tile([6, 64], F32, tag="wcd1")
            wb1 = ecp.tile([64, 64], F32, tag="wb1")
            nc.sync.dma_start(out=wd1, in_=ins["wd1T"])
            nc.sync.dma_start(out=wcd1, in_=ins["wcd1T"])
            nc.sync.dma_start(out=wb1, in_=ins["wb1T"])
            gb1a = ecp.tile([64, 2], F32, tag="gb1a")
            gb1b = ecp.tile([64, 2], F32, tag="gb1b")
            nc.sync.dma_start(out=gb1a, in_=ins["gb1a"])
            nc.sync.dma_start(out=gb1b, in_=ins["gb1b"])
            build_tables(ecp, augR[:], 6, wd1[:], wcd1[:], at_d[0], bvT)
            sc, sh = edgeconv_passA(0, ecp, augL, augR, 6, at_d[0], stg_d[0], gb1a[:],
                                    enc_s4096=float(32 * 4096))
            edgeconv_passB(0, ecp, stg_d[0], wb1[:], sc, sh, gb1b[:], x12[0:64, :])
            if dbg is not None:
                nc.sync.dma_start(out=dbg["x1"], in_=x12[0:64, :])

            # ================= EC2 =================
            wd2 = ecp.tile([64, 64], F32, tag="wd2")
            wcd2 = ecp.tile([64, 64], F32, tag="wcd2")
            wb2 = ecp.tile([64, 64], F32, tag="wb2")
            nc.sync.dma_start(out=wd2, in_=ins["wd2T"])
            nc.sync.dma_start(out=wcd2, in_=ins["wcd2T"])
            nc.sync.dma_start(out=wb2, in_=ins["wb2T"])
            gb2a = ecp.tile([64, 2], F32, tag="gb2a")
            gb2b = ecp.tile([64, 2], F32, tag="gb2b")
            nc.sync.dma_start(out=gb2a, in_=ins["gb2a"])
            nc.sync.dma_start(out=gb2b, in_=ins["gb2b"])
            augL2, augR2 = build_aug(ecp, x12[0:64, :], 64)
            build_tables(ecp, augR2[:], 64, wd2[:], wcd2[:], at_d[1], bvT)
            sc, sh = edgeconv_passA(1, ecp, augL2, augR2, 64, at_d[1], stg_d[1], gb2a[:],
                                    enc_s4096=float(2 * 4096))
            edgeconv_passB(1, ecp, stg_d[1], wb2[:], sc, sh, gb2b[:], x2t[:])
            nc.sync.dma_start(out=x12[64:128, :], in_=x2t[:])
            if dbg is not None:
                nc.sync.dma_start(out=dbg["x2"], in_=x2t[:])

            # ================= EC3 =================
            wd3 = ecp.tile([64, 64], F32, tag="wd3")
            wcd3 = ecp.tile([64, 64], F32, tag="wcd3")
            nc.sync.dma_start(out=wd3, in_=ins["wd3T"])
            nc.sync.dma_start(out=wcd3, in_=ins["wcd3T"])
            gb3 = ecp.tile([64, 2], F32, tag="gb3")
            nc.sync.dma_start(out=gb3, in_=ins["gb3"])
            augL3, augR3 = build_aug(ecp, x2t[:], 64)
            build_tables(ecp, augR3[:], 64, wd3[:], wcd3[:], at_d[2], bvT)
            sc, sh = edgeconv_passA(2, ecp, augL3, augR3, 64, at_d[2], None, gb3[:],
                                    enc_s4096=float(2 * 4096))
            emit_lrelu(ecp, x3t[:], maxE3[:], sc, sh, tag="lrX3")
            if dbg is not None:
                nc.sync.dma_start(out=dbg["x3"], in_=x3t[:])

        # ================= head =================
        with tc.tile_pool(name="head", bufs=1) as hp, \
             tc.tile_pool(name="headp", bufs=2, space="PSUM") as hpp:
            w4a = hp.tile([P, 1024], F32, tag="w4a")
            w4b = hp.tile([64, 1024], F32, tag="w4b")
            nc.sync.dma_start(out=w4a, in_=ins["w4Ta"])
            nc.sync.dma_start(out=w4b, in_=ins["w4Tb"])
            gb4 = hp.tile([P, 16], F32, tag="gb4")
            nc.sync.dma_start(out=gb4, in_=ins["gb4"])

            # conv4: stats + max over N, never materialize h4
            s123a = hp.tile([P, 1], F32, tag="s123a")
            s123b = hp.tile([64, 1], F32, tag="s123b")
            nc.vector.tensor_reduce(out=s123a, in_=x12[:], op=ALU.add, axis=AX.X)
            nc.vector.tensor_reduce(out=s123b, in_=x3t[:], op=ALU.add, axis=AX.X)
            st4 = hp.tile([P, 16], F32, tag="st4")      # (sum, sq) x 8 groups
            mx4 = hp.tile([P, 8], F32, tag="mx4")
            scr4 = hp.tile([P, 512], F32, tag="scr4")
            sq4acc = hp.tile([P, 8 * NC512], F32, tag="sq4acc")
            for g in range(8):
                sp4 = hpp.tile([P, 1], F32, tag="sp")
                nc.tensor.matmul(sp4, lhsT=w4a[:, g * P:(g + 1) * P], rhs=s123a[:],
                                 start=True, stop=False)
                nc.tensor.matmul(sp4, lhsT=w4b[:, g * P:(g + 1) * P], rhs=s123b[:],
                                 start=False, stop=True)
                nc.scalar.copy(st4[:, 2 * g:2 * g + 1], sp4)
                for ch in range(NC512):
                    hps = hpp.tile([P, 512], F32, tag="hps")
                    nc.tensor.matmul(hps, lhsT=r32(w4a[:, g * P:(g + 1) * P]),
                                     rhs=r32(x12[:, ch * 512:(ch + 1) * 512]),
                                     start=True, stop=False)
                    nc.tensor.matmul(hps, lhsT=r32(w4b[:, g * P:(g + 1) * P]),
                                     rhs=r32(x3t[:, ch * 512:(ch + 1) * 512]),
                                     start=False, stop=True)
                    nc.scalar.activation(scr4, hps, AF.Square,
                                         accum_out=sq4acc[:, g * NC512 + ch:g * NC512 + ch + 1])
                    mxc = hp.tile([P, 1], F32, tag="mxc")
                    nc.vector.tensor_reduce(out=mxc, in_=hps[:], op=ALU.max, axis=AX.X)
                    if ch == 0:
                        nc.vector.tensor_copy(mx4[:, g:g + 1], mxc)
                    else:
                        nc.vector.tensor_tensor(mx4[:, g:g + 1], mx4[:, g:g + 1], mxc, op=ALU.max)
            for g in range(8):
                nc.vector.tensor_reduce(out=st4[:, 2 * g + 1:2 * g + 2],
                                        in_=sq4acc[:, g * NC512:(g + 1) * NC512],
                                        op=ALU.add, axis=AX.X)
            allreduce_stats(st4[:], P, 16)
            sc4, sh4 = bn_coeffs(hp, st4[:], gb4[:], CNT_N, P, ncols=8)
            # g4 = lrelu(bn4(max)) : elementwise on [P, 8]
            g4 = hp.tile([P, 8], F32, tag="g4")
            nc.vector.tensor_tensor(g4, mx4, sc4, op=ALU.mult)
            nc.vector.tensor_add(g4, g4, sh4)
            g4n = hp.tile([P, 8], F32, tag="g4n")
            nc.vector.tensor_scalar_mul(g4n, g4, 0.2)
            nc.vector.tensor_tensor(g4, g4, g4n, op=ALU.max)

            # conv5: y5 = W5x @ x123 + (W5g @ g4)
            w5xa = hp.tile([P, 256], F32, tag="w5xa")
            w5xb = hp.tile([64, 256], F32, tag="w5xb")
            nc.sync.dma_start(out=w5xa, in_=ins["w5xTa"])
            nc.sync.dma_start(out=w5xb, in_=ins["w5xTb"])
            gb5 = hp.tile([P, 4], F32, tag="gb5")
            nc.sync.dma_start(out=gb5, in_=ins["gb5"])
            c5 = hp.tile([P, 2], F32, tag="c5")
            with tc.tile_pool(name="w5g", bufs=2) as w5p:
                for og in range(2):
                    c5p = hpp.tile([P, 1], F32, tag="sp")
                    for kc in range(8):
                        w5g = w5p.tile([P, P], F32, tag="w5g")
                        nc.sync.dma_start(out=w5g, in_=ins["w5gT"][kc * P:(kc + 1) * P,
                                                                   og * P:(og + 1) * P])
                        nc.tensor.matmul(c5p, lhsT=w5g[:], rhs=g4[:, kc:kc + 1],
                                         start=(kc == 0), stop=(kc == 7))
                    nc.scalar.copy(c5[:, og:og + 1], c5p)

            h5 = hp.tile([P, 2 * N], F32, tag="h5")
            sq5acc = hp.tile([P, 2 * NC512], F32, tag="sq5acc")
            st5 = hp.tile([P, 4], F32, tag="st5")
            scr5 = hp.tile([P, 512], F32, tag="scr5")
            for og in range(2):
                # sum: W5x @ s123 + N * c5
                sp5 = hpp.tile([P, 1], F32, tag="sp")
                nc.tensor.matmul(sp5, lhsT=w5xa[:, og * P:(og + 1) * P], rhs=s123a[:],
                                 start=True, stop=False)
                nc.tensor.matmul(sp5, lhsT=w5xb[:, og * P:(og + 1) * P], rhs=s123b[:],
                                 start=False, stop=True)
                sc_t = hp.tile([P, 1], F32, tag="sc_t")
                nc.vector.tensor_scalar_mul(sc_t, c5[:, og:og + 1], float(N))
                nc.vector.tensor_copy(st5[:, 2 * og:2 * og + 1], sp5)
                nc.vector.tensor_add(st5[:, 2 * og:2 * og + 1],
                                     st5[:, 2 * og:2 * og + 1], sc_t)
                for ch in range(NC512):
                    hps = hpp.tile([P, 512], F32, tag="hps")
                    nc.tensor.matmul(hps, lhsT=r32(w5xa[:, og * P:(og + 1) * P]),
                                     rhs=r32(x12[:, ch * 512:(ch + 1) * 512]),
                                     start=True, stop=False)
                    nc.tensor.matmul(hps, lhsT=r32(w5xb[:, og * P:(og + 1) * P]),
                                     rhs=r32(x3t[:, ch * 512:(ch + 1) * 512]),
                                     start=False, stop=True)
                    dst = h5[:, og * N + ch * 512: og * N + (ch + 1) * 512]
                    nc.vector.tensor_scalar(dst, hps, c5[:, og:og + 1], None, op0=ALU.add)
                    nc.scalar.activation(scr5, dst, AF.Square,
                                         accum_out=sq5acc[:, og * NC512 + ch:og * NC512 + ch + 1])
            for og in range(2):
                nc.vector.tensor_reduce(out=st5[:, 2 * og + 1:2 * og + 2],
                                        in_=sq5acc[:, og * NC512:(og + 1) * NC512],
                                        op=ALU.add, axis=AX.X)
            allreduce_stats(st5[:], P, 4)
            sc5, sh5 = bn_coeffs(hp, st5[:], gb5[:], CNT_N, P, ncols=2)
            for og in range(2):
                emit_lrelu(hp, h5[:, og * N:(og + 1) * N], h5[:, og * N:(og + 1) * N],
                           sc5[:, og:og + 1], sh5[:, og:og + 1], tag="lrH")

            # conv6: 256 -> 256
            w6 = hp.tile([P, 2 * 256], F32, tag="w6")     # [256,256].T split: rows kc
            nc.sync.dma_start(out=w6, in_=ins["w6T"])     # host packs [128, 512]
            gb6 = hp.tile([P, 4], F32, tag="gb6")
            nc.sync.dma_start(out=gb6, in_=ins["gb6"])
            h6 = hp.tile([P, 2 * N], F32, tag="h6")
            sq6acc = hp.tile([P, 2 * NC512], F32, tag="sq6acc")
            st6 = hp.tile([P, 4], F32, tag="st6")
            s5 = hp.tile([P, 2], F32, tag="s5")
            for og in range(2):
                nc.vector.tensor_reduce(out=s5[:, og:og + 1], in_=h5[:, og * N:(og + 1) * N],
                                        op=ALU.add, axis=AX.X)
            for og in range(2):
                sp6 = hpp.tile([P, 1], F32, tag="sp")
                for kc in range(2):
                    nc.tensor.matmul(sp6, lhsT=w6[:, kc * 256 + og * P: kc * 256 + (og + 1) * P],
                                     rhs=s5[:, kc:kc + 1], start=(kc == 0), stop=(kc == 1))
                nc.scalar.copy(st6[:, 2 * og:2 * og + 1], sp6)
                for ch in range(NC512):
                    hps = hpp.tile([P, 512], F32, tag="hps")
                    for kc in range(2):
                        nc.tensor.matmul(hps,
                                         lhsT=r32(w6[:, kc * 256 + og * P: kc * 256 + (og + 1) * P]),
                                         rhs=r32(h5[:, kc * N + ch * 512: kc * N + (ch + 1) * 512]),
                                         start=(kc == 0), stop=(kc == 1))
                    dst = h6[:, og * N + ch * 512: og * N + (ch + 1) * 512]
                    nc.scalar.activation(scr5, hps, AF.Square,
                                         accum_out=sq6acc[:, og * NC512 + ch:og * NC512 + ch + 1])
                    nc.vector.tensor_copy(dst, hps)
            for og in range(2):
                nc.vector.tensor_reduce(out=st6[:, 2 * og + 1:2 * og + 2],
                                        in_=sq6acc[:, og * NC512:(og + 1) * NC512],
                                        op=ALU.add, axis=AX.X)
            allreduce_stats(st6[:], P, 4)
            sc6, sh6 = bn_coeffs(hp, st6[:], gb6[:], CNT_N, P, ncols=2)
            for og in range(2):
                emit_lrelu(hp, h6[:, og * N:(og + 1) * N], h6[:, og * N:(og + 1) * N],
                           sc6[:, og:og + 1], sh6[:, og:og + 1], tag="lrH")

            # conv7: 256 -> 128
            w7 = hp.tile([P, 2 * P], F32, tag="w7")       # [256,128].T: two [128,128]
            nc.sync.dma_start(out=w7, in_=ins["w7T"])
            gb7 = hp.tile([P, 2], F32, tag="gb7")
            nc.sync.dma_start(out=gb7, in_=ins["gb7"])
            h7 = hp.tile([P, N], F32, tag="h7")
            sq7acc = hp.tile([P, NC512], F32, tag="sq7acc")
            st7 = hp.tile([P, 2], F32, tag="st7")
            s6 = hp.tile([P, 2], F32, tag="s6")
            for og in range(2):
                nc.vector.tensor_reduce(out=s6[:, og:og + 1], in_=h6[:, og * N:(og + 1) * N],
                                        op=ALU.add, axis=AX.X)
            sp7 = hpp.tile([P, 1], F32, tag="sp")
            for kc in range(2):
                nc.tensor.matmul(sp7, lhsT=w7[:, kc * P:(kc + 1) * P], rhs=s6[:, kc:kc + 1],
                                 start=(kc == 0), stop=(kc == 1))
            nc.scalar.copy(st7[:, 0:1], sp7)
            for ch in range(NC512):
                hps = hpp.tile([P, 512], F32, tag="hps")
                for kc in range(2):
                    nc.tensor.matmul(hps, lhsT=r32(w7[:, kc * P:(kc + 1) * P]),
                                     rhs=r32(h6[:, kc * N + ch * 512: kc * N + (ch + 1) * 512]),
                                     start=(kc == 0), stop=(kc == 1))
                dst = h7[:, ch * 512:(ch + 1) * 512]
                nc.scalar.activation(scr5, hps, AF.Square,
                                     accum_out=sq7acc[:, ch:ch + 1])
                nc.vector.tensor_copy(dst, hps)
            nc.vector.tensor_reduce(out=st7[:, 1:2], in_=sq7acc[:], op=ALU.add, axis=AX.X)
            allreduce_stats(st7[:], P, 2)
            sc7, sh7 = bn_coeffs(hp, st7[:], gb7[:], CNT_N, P, ncols=1)
            emit_lrelu(hp, h7[:], h7[:], sc7, sh7, tag="lrH")

            # conv8: 128 -> 6, no bn/act
            w8 = hp.tile([P, 6], F32, tag="w8")
            nc.sync.dma_start(out=w8, in_=ins["w8T"])
            osb = hp.tile([6, N], F32, tag="osb")
            for ch in range(NC512):
                ops = hpp.tile([6, 512], F32, tag="hps")
                nc.tensor.matmul(ops, lhsT=r32(w8[:]), rhs=r32(h7[:, ch * 512:(ch + 1) * 512]),
                                 start=True, stop=True)
                nc.scalar.copy(osb[:, ch * 512:(ch + 1) * 512], ops)
            nc.sync.dma_start(out=out_dram, in_=osb)
        es.close()
    assert n_ar == 9, n_ar


def prep_inputs(inputs, N):
    """Host-side weight/shape prep. Returns per-core in_maps (core c gets
    cloud c) given the full input dict from setup_inputs()."""
    f = {k: np.asarray(v, dtype=np.float32) for k, v in inputs.items()}
    x = f["x"]                            # [8, 6, N]
    B = x.shape[0]

    def gbpair(g, b):
        out = np.stack([g, b], axis=1).astype(np.float32)   # [ch, 2]
        return out

    def gbgrp(g, b, ngr):
        # [ngr*128] channels -> [128, 2*ngr] (g,b) interleaved per group
        out = np.zeros((P, 2 * ngr), np.float32)
        for gi in range(ngr):
            out[:, 2 * gi] = g[gi * P:(gi + 1) * P]
            out[:, 2 * gi + 1] = b[gi * P:(gi + 1) * P]
        return out

    w1a, w1b = f["w1a"], f["w1b"]
    w2a, w2b = f["w2a"], f["w2b"]
    w3, w4, w5, w6, w7, w8 = f["w3"], f["w4"], f["w5"], f["w6"], f["w7"], f["w8"]
    shared = {
        "wd1T": np.ascontiguousarray(w1a[:, :6].T),
        "wcd1T": np.ascontiguousarray((w1a[:, 6:] - w1a[:, :6]).T),
        "wb1T": np.ascontiguousarray(w1b.T),
        "wd2T": np.ascontiguousarray(w2a[:, :64].T),
        "wcd2T": np.ascontiguousarray((w2a[:, 64:] - w2a[:, :64]).T),
        "wb2T": np.ascontiguousarray(w2b.T),
        "wd3T": np.ascontiguousarray(w3[:, :64].T),
        "wcd3T": np.ascontiguousarray((w3[:, 64:] - w3[:, :64]).T),
        "gb1a": gbpair(f["g1a"], f["b1a"]),
        "gb1b": gbpair(f["g1b"], f["b1b"]),
        "gb2a": gbpair(f["g2a"], f["b2a"]),
        "gb2b": gbpair(f["g2b"], f["b2b"]),
        "gb3": gbpair(f["g3"], f["b3"]),
        "w4Ta": np.ascontiguousarray(w4.T[:128, :]),
        "w4Tb": np.ascontiguousarray(w4.T[128:, :]),
        "gb4": gbgrp(f["g4"], f["b4"], 8),
        "w5xTa": np.ascontiguousarray(w5[:, 1024:].T[:128, :]),
        "w5xTb": np.ascontiguousarray(w5[:, 1024:].T[128:, :]),
        "w5gT": np.ascontiguousarray(w5[:, :1024].T),
        "gb5": gbgrp(f["g5"], f["b5"], 2),
        # w6T packed [128, 2*256]: kc-th K-chunk of w6.T at cols kc*256
        "w6T": np.concatenate([w6.T[:128, :], w6.T[128:, :]], axis=1),
        "gb6": gbgrp(f["g6"], f["b6"], 2),
        "w7T": np.concatenate([w7.T[:128, :], w7.T[128:, :]], axis=1),
        "gb7": gbgrp(f["g7"], f["b7"], 1),
        "w8T": np.ascontiguousarray(w8.T),
    }
    shared = {k: np.ascontiguousarray(v, dtype=np.float32) for k, v in shared.items()}
    in_maps = []
    for c in range(B):
        m = dict(shared)
        m["x"] = np.ascontiguousarray(x[c])
        in_maps.append(m)
    return in_maps


_CACHED = {}


def kernel(**inputs) -> np.ndarray:
    from concourse.bass_utils import run_bass_kernel_spmd
    N = int(np.asarray(inputs["x"]).shape[2])
    n_cores = 8
    in_maps = prep_inputs(inputs, N)
    key = (N, n_cores)
    if key not in _CACHED:
        nc = bacc.Bacc("TRN2", target_bir_lowering=False, debug=False,
                       num_devices=n_cores)
        ins = {}
        for k, v in in_maps[0].items():
            ins[k] = nc.dram_tensor(k, list(v.shape), F32, kind="ExternalInput").ap()
        outs = {"out": nc.dram_tensor("out", [6, N], F32, kind="ExternalOutput").ap()}
        build_graph(nc, ins, outs, N, n_cores, lrelu_native=False)
        nc.compile()
        _CACHED[key] = nc
    nc = _CACHED[key]
    res = run_bass_kernel_spmd(nc, in_maps, core_ids=list(range(n_cores)))
    out = np.stack([res.results[c]["out"] for c in range(n_cores)], axis=0)
    return out.astype(np.float32)


def kernel_traced(**inputs):
    """Like kernel() but captures the NTFF profile; returns (out, exec_ns)."""
    from concourse.bass_utils import run_bass_kernel_spmd
    N = int(np.asarray(inputs["x"]).shape[2])
    n_cores = 8
    in_maps = prep_inputs(inputs, N)
    key = (N, n_cores)
    if key not in _CACHED:
        kernel(**inputs)
    nc = _CACHED[key]
    res = run_bass_kernel_spmd(nc, in_maps, core_ids=list(range(n_cores)),
                               trace=True)
    out = np.stack([res.results[c]["out"] for c in range(n_cores)], axis=0)
    return out.astype(np.float32), res.exec_time_ns



# revision 12
# speedup vs baseline: 1.0248x; 1.0248x over previous
"""DGCNN segmentation forward pass on 8 Trainium2 NeuronCores.

Data-parallel over batch: core c processes cloud c. Training-mode BatchNorm
statistics couple the batch, so each BN layer does a tiny cross-core
AllReduce of per-channel (sum, sumsq) between compute phases.

Self-contained: hardcodes B=8, C=6, N=4096, K=20.
"""
import numpy as np

import concourse.bacc as bacc
import concourse.bass as bass
import concourse.mybir as mybir
from concourse.tile import TileContext
from concourse.masks import make_identity

P = 128
K = 20
EPS = 1e-5
BIG = float(3 * 2 ** 34)   # 1.5*2^35: signed round-to-4096-grid magic bias
CREC = float(3 * 2 ** 22)  # 1.5*2^23: signed round-to-int magic (index recovery)
F32 = mybir.dt.float32
F32R = mybir.dt.float32r
U32 = mybir.dt.uint32
AF = mybir.ActivationFunctionType
ALU = mybir.AluOpType
AX = mybir.AxisListType


import os
USE_F32R_DIST = os.environ.get("F32R_DIST", "0") == "1"
USE_F32R_CONV = os.environ.get("F32R_CONV", "0") == "1"
BATCH_GATHER = os.environ.get("BATCH_GATHER", "1") == "1"
BCAST_BV = os.environ.get("BCAST_BV", "1") == "1"


def bcast_mid(ap, k):
    """[P, C] AP -> [P, k, C] AP with zero-stride broadcast middle dim."""
    return bass.AP(ap.tensor, ap.offset, [ap.ap[0], [0, k], ap.ap[1]])


def r32(ap):
    return ap.bitcast(F32R) if USE_F32R_CONV else ap


def r32d(ap):
    return ap.bitcast(F32R) if USE_F32R_DIST else ap


def build_graph(nc, ins, outs, N, n_cores, lrelu_native=True, dbg=None,
                skip_collectives=False):
    """Emit the whole forward pass. ins/outs: dicts of DRAM APs."""
    NT = N // P
    NC512 = N // 512
    CNT_E = float(n_cores * N * K)   # BN sample count, conv2d layers
    CNT_N = float(n_cores * N)       # BN sample count, conv1d layers
    RG = [list(range(n_cores))]

    x_in = ins["x"]                  # [6, N]
    out_dram = outs["out"]           # [6, N]

    # scratch DRAM
    at_d = [nc.dram_tensor(f"at{i}", [N, 64], F32, kind="Internal") for i in range(3)]
    iwd_d = [nc.dram_tensor(f"iwd{i}", [16, (N // P) * K * 8], mybir.dt.int16,
                            kind="Internal") for i in range(3)]
    # slab layout: tile t occupies rows [t*64, (t+1)*64) -> contiguous 640KB
    # per tile so the DMA sprays across all 16 engines.
    stg_d = [nc.dram_tensor(f"stg{i}", [NT * 64, K * P], F32, kind="Internal") for i in range(2)]
    n_ar = 0

    with TileContext(nc) as tc:
        from contextlib import ExitStack
        es = ExitStack()
        lp = es.enter_context(tc.tile_pool(name="long", bufs=1))

        ident = lp.tile([P, P], F32)
        make_identity(nc, ident[:])
        ones_col = lp.tile([P, 1], F32)
        nc.vector.memset(ones_col, 1.0)

        x12 = lp.tile([P, N], F32)       # x1 (rows 0:64) and x2 (rows 64:128)
        x3t = lp.tile([64, N], F32)
        maxY = lp.tile([64, N], F32)     # per-EC max-over-k staging (also EC3)
        maxE3 = maxY
        x2t = lp.tile([64, N], F32)      # base-0 copy of x2 (engines cannot
                                         # write across partition bases)

        # ---------- helpers ----------
        def allreduce_stats(st_ap, rows, cols):
            """st_ap: SBUF [rows, cols] -> allreduced in place."""
            nonlocal n_ar
            i = n_ar
            n_ar += 1
            cci = nc.dram_tensor(f"cci{i}", [rows, cols], F32, kind="Internal")
            cco = nc.dram_tensor(f"cco{i}", [rows, cols], F32, kind="Internal",
                                 addr_space="Shared")
            nc.sync.dma_start(out=cci.ap(), in_=st_ap)
            if not skip_collectives:
                nc.gpsimd.collective_compute(
                    kind="AllReduce", op=ALU.add, replica_groups=RG,
                    ins=[cci.ap()], outs=[cco.ap()])
                nc.sync.dma_start(out=st_ap, in_=cco.ap())

        def bn_coeffs(pool, st, gb, cnt, rows, ncols=1):
            """st: [rows, 2*ncols] (sum, sumsq pairs), gb: [rows, 2*ncols]
            (g, b pairs). Returns scale/shift [rows, ncols] tiles."""
            mean = pool.tile([rows, ncols], F32, tag="bn_mean")
            ex2 = pool.tile([rows, ncols], F32, tag="bn_ex2")
            scale = pool.tile([rows, ncols], F32, tag="bn_scale")
            shift = pool.tile([rows, ncols], F32, tag="bn_shift")
            s_cols = st[:, 0:2 * ncols:2]
            q_cols = st[:, 1:2 * ncols:2]
            nc.vector.tensor_scalar_mul(mean, s_cols, 1.0 / cnt)
            nc.vector.tensor_scalar_mul(ex2, q_cols, 1.0 / cnt)
            msq = pool.tile([rows, ncols], F32, tag="bn_msq")
            nc.vector.tensor_tensor(msq, mean, mean, op=ALU.mult)
            nc.vector.tensor_sub(ex2, ex2, msq)          # var
            std = pool.tile([rows, ncols], F32, tag="bn_std")
            epst = pool.tile([rows, 1], F32, tag="bn_eps")
            nc.vector.memset(epst, EPS)
            nc.scalar.activation(std, ex2, AF.Sqrt, bias=epst[:])
            nc.vector.reciprocal(scale, std)             # 1/std
            nc.vector.tensor_tensor(scale, scale, gb[:, 0:2 * ncols:2], op=ALU.mult)
            nc.vector.tensor_tensor(msq, mean, scale, op=ALU.mult)
            nc.vector.tensor_sub(shift, gb[:, 1:2 * ncols:2], msq)
            return scale[:], shift[:]

        def emit_lrelu(pool, out_ap, in_ap, scale, bias, accum_ap=None, tag="lr"):
            """out = lrelu(in*scale + bias); optional accum_ap [P,1] = row sums
            of the OUTPUT."""
            if lrelu_native:
                kw = {"accum_out": accum_ap} if accum_ap is not None else {}
                nc.scalar.activation(out_ap, in_ap, AF.Lrelu, bias=bias, scale=scale,
                                     alpha=0.2, **kw)
            else:
                nc.scalar.activation(out_ap, in_ap, AF.Identity, bias=bias, scale=scale)
                t = pool.tile([out_ap.shape[0], out_ap.free_size()], F32, tag=tag)
                nc.vector.tensor_scalar(t[:], out_ap, 0.2, None, op0=ALU.mult)
                nc.vector.tensor_tensor(out_ap, out_ap, t[:], op=ALU.max)
                if accum_ap is not None:
                    nc.vector.tensor_reduce(out=accum_ap, in_=out_ap, op=ALU.add,
                                            axis=AX.X)

        def build_aug(pool, src_ap, cin, from_dram=False):
            """Return (augL [cin+1, N] = [2x;1], augR [cin+1, N] = [x;-|x|^2])."""
            augL = pool.tile([cin + 1, N], F32, tag="augL")
            augR = pool.tile([cin + 1, N], F32, tag="augR")
            if from_dram:
                nc.sync.dma_start(out=augR[:cin, :], in_=src_ap)
            else:
                nc.scalar.copy(augR[:cin, :], src_ap)
            nc.scalar.activation(augL[:cin, :], augR[:cin, :], AF.Copy, bias=0.0,
                                 scale=2.0)
            onesrow = pool.tile([1, 512], F32, tag="onesrow")
            nc.vector.memset(onesrow, 1.0)
            with tc.tile_pool(name="sqp", bufs=2, space="PSUM") as sqp, \
                 tc.tile_pool(name="sqs", bufs=2) as sqs:
                for ch in range(NC512):
                    sl = slice(ch * 512, (ch + 1) * 512)
                    xsq = sqs.tile([cin, 512], F32, tag="xsq")
                    nc.scalar.activation(xsq, augR[:cin, sl], AF.Square)
                    ps = sqp.tile([1, 512], F32, tag="sq_ps")
                    nc.tensor.matmul(ps, lhsT=ones_col[:cin, :], rhs=xsq[:],
                                     start=True, stop=True)
                    srow = sqs.tile([1, 512], F32, tag="srow")
                    nc.scalar.activation(srow, ps, AF.Copy, bias=0.0, scale=-1.0)
                    nc.sync.dma_start(out=augR[cin:cin + 1, sl], in_=srow[:])
                    nc.sync.dma_start(out=augL[cin:cin + 1, sl], in_=onesrow[:])
            if dbg is not None:
                nc.sync.dma_start(out=dbg[f"srow{cin}"], in_=augR[cin:cin + 1, :])
            return augL, augR

        def build_tables(pool, augR, cin, wdT, wcdT, at_dram, bvT):
            """AT table -> DRAM, BvT -> SBUF [P, NT*64]."""
            with tc.tile_pool(name="tbl", bufs=3, space="PSUM") as tp, \
                 tc.tile_pool(name="tbls", bufs=3) as tsb:
                for t in range(NT):
                    xc = augR[:cin, t * P:(t + 1) * P]
                    ps_a = tp.tile([P, 64], F32, tag="at_ps")
                    nc.tensor.matmul(ps_a, lhsT=xc, rhs=wdT, start=True, stop=True)
                    a_sb = tsb.tile([P, 64], F32, tag="at_sb")
                    nc.scalar.copy(a_sb, ps_a)
                    nc.sync.dma_start(out=at_dram.ap()[t * P:(t + 1) * P, :], in_=a_sb)
                    ps_b = tp.tile([P, 64], F32, tag="bv_ps")
                    nc.tensor.matmul(ps_b, lhsT=xc, rhs=wcdT, start=True, stop=True)
                    nc.scalar.copy(bvT[:, t * 64:(t + 1) * 64], ps_b)

        def edgeconv_passA(ec, pool, augL, augR, cin, at_dram, stg, gb_a,
                           enc_s4096=8192.0):
            sumA = pool.tile([P, 64], F32, tag="sumA")
            sqA = pool.tile([P, 64], F32, tag="sqA")
            nc.vector.memset(sumA, 0.0)
            nc.vector.memset(sqA, 0.0)
            with tc.tile_pool(name=f"pa{ec}", bufs=2) as wp, \
                 tc.tile_pool(name=f"pap{ec}", bufs=2, space="PSUM") as pp:
                for t in range(NT):
                    dtile = wp.tile([P, N], F32, tag="dtile")
                    for ch in range(NC512):
                        dps = pp.tile([P, 512], F32, tag="dps")
                        nc.tensor.matmul(dps, lhsT=r32d(augL[:, t * P:(t + 1) * P]),
                                         rhs=r32d(augR[:, ch * 512:(ch + 1) * 512]),
                                         start=True, stop=True)
                        nc.scalar.copy(dtile[:, ch * 512:(ch + 1) * 512], dps)
                    vals = wp.tile([P, 24], F32, tag="vals")
                    idxs = wp.tile([P, 24], U32, tag="idxs")
                    CH = N // 32
                    cand = wp.tile([P, 256], F32, tag="cand")
                    for c_ in range(32):
                        nc.vector.max(out=cand[:, 8 * c_:8 * c_ + 8],
                                      in_=dtile[:, c_ * CH:(c_ + 1) * CH])
                    for r in range(3):
                        v8 = vals[:, 8 * r:8 * r + 8]
                        nc.vector.max(out=v8, in_=cand[:])
                        if r < 2:
                            nc.vector.match_replace(out=cand[:], in_to_replace=v8,
                                                    in_values=cand[:], imm_value=-1e30)
                    for r in range(3):
                        nc.vector.max_index(out=idxs[:, 8 * r:8 * r + 8],
                                            in_max=vals[:, 8 * r:8 * r + 8],
                                            in_values=dtile[:])
                    if dbg is not None and t == 0:
                        nc.sync.dma_start(out=dbg[f"idx{ec}"], in_=idxs[:])
                        nc.sync.dma_start(out=dbg[f"vals{ec}"], in_=vals[:])
                    ep = wp.tile([P, K * 64], F32, tag="ep")
                    if BATCH_GATHER:
                        nc.gpsimd.indirect_dma_start(
                            out=ep[:, 0:K * 64], out_offset=None,
                            in_=at_dram.ap(),
                            in_offset=bass.IndirectOffsetOnAxis(
                                ap=idxs[:, 0:K], axis=0))
                    else:
                        for s_ in range(K):
                            nc.gpsimd.indirect_dma_start(
                                out=ep[:, s_ * 64:(s_ + 1) * 64], out_offset=None,
                                in_=at_dram.ap(),
                                in_offset=bass.IndirectOffsetOnAxis(
                                    ap=idxs[:, s_:s_ + 1], axis=0))
                    if dbg is not None and t == 0:
                        nc.sync.dma_start(out=dbg[f"graw{ec}"], in_=ep[:])
                    if BCAST_BV:
                        ep_v = ep[:, 0:K * 64].rearrange("p (k c) -> p k c", k=K)
                        bv_b = bcast_mid(bvT[:, t * 64:(t + 1) * 64], K)
                        nc.vector.tensor_tensor(ep_v, ep_v, bv_b, op=ALU.add)
                    else:
                        for s_ in range(K):
                            nc.vector.tensor_add(
                                ep[:, s_ * 64:(s_ + 1) * 64],
                                ep[:, s_ * 64:(s_ + 1) * 64],
                                bvT[:, t * 64:(t + 1) * 64])
                    if dbg is not None and t == 0:
                        nc.sync.dma_start(out=dbg[f"ep{ec}"], in_=ep[:])
                    red = wp.tile([P, 64], F32, tag="red")
                    nc.vector.tensor_reduce(out=red, in_=ep[:].rearrange("p (k c) -> p c k", k=K),
                                            op=ALU.add, axis=AX.X)
                    nc.vector.tensor_add(sumA, sumA, red)
                    sq = wp.tile([P, K * 64], F32, tag="eT")
                    nc.scalar.activation(sq, ep[:], AF.Square)
                    nc.vector.tensor_reduce(out=red, in_=sq[:].rearrange("p (k c) -> p c k", k=K),
                                            op=ALU.add, axis=AX.X)
                    nc.vector.tensor_add(sqA, sqA, red)
                    if ec < 2:
                        eT = wp.tile([64, K * P], F32, tag="eT")
                        for blk in range(5):
                            tps = pp.tile([64, 512], F32, tag="tps")
                            for s4 in range(4):
                                s = blk * 4 + s4
                                nc.tensor.transpose(tps[:, s4 * P:(s4 + 1) * P],
                                                    ep[:, s * 64:(s + 1) * 64], ident[:])
                            nc.scalar.copy(eT[:, blk * 512:(blk + 1) * 512], tps)
                        nc.sync.dma_start(out=stg.ap()[t * 64:(t + 1) * 64, :], in_=eT)
                    else:
                        m3 = wp.tile([P, 64], F32, tag="m3")
                        nc.vector.tensor_reduce(out=m3, in_=ep[:].rearrange("p (k c) -> p c k", k=K),
                                                op=ALU.max, axis=AX.X)
                        mps = pp.tile([64, P], F32, tag="mps")
                        nc.tensor.transpose(mps, m3[:], ident[:])
                        nc.scalar.copy(maxE3[:, t * P:(t + 1) * P], mps)
                stp = pp.tile([64, 2], F32, tag="stp")
                nc.tensor.matmul(stp[:, 0:1], lhsT=sumA[:], rhs=ones_col[:], start=True, stop=True)
                nc.tensor.matmul(stp[:, 1:2], lhsT=sqA[:], rhs=ones_col[:], start=True, stop=True)
                st = pool.tile([64, 2], F32, tag=f"st_a{ec}")
                nc.scalar.copy(st, stp)
            allreduce_stats(st[:], 64, 2)
            if dbg is not None:
                nc.sync.dma_start(out=dbg[f"sta{ec}"], in_=st[:])
            return bn_coeffs(pool, st[:], gb_a, CNT_E, 64)

        def edgeconv_passB(ec, pool, stg, wbT, sc_a, sh_a, gb_b, x_dst):
            """load staged e_preT, bn+lrelu, conv-b, max-over-k, stats; then
            allreduce + write x_dst = lrelu(bn_b(maxY))."""
            accS = pool.tile([64, NT], F32, tag="accS")      # sum of e1 per tile
            sqB = pool.tile([64, NT * 5], F32, tag="sqB")    # sumsq of y per chunk
            with tc.tile_pool(name=f"pb{ec}", bufs=2) as wp, \
                 tc.tile_pool(name=f"pbp{ec}", bufs=2, space="PSUM") as pp:
                for t in range(NT):
                    eT = wp.tile([64, K * P], F32, tag="eTb")
                    nc.sync.dma_start(out=eT, in_=stg.ap()[t * 64:(t + 1) * 64, :])
                    e1 = eT   # lrelu applied in place to save SBUF
                    emit_lrelu(wp, e1[:], eT[:], sc_a, sh_a, accum_ap=accS[:, t:t + 1],
                               tag="lrB")
                    scr = wp.tile([64, 512], F32, tag="scr")
                    for ch in range(5):
                        cps = pp.tile([64, 512], F32, tag="cps")
                        nc.tensor.matmul(cps, lhsT=r32(wbT), rhs=r32(e1[:, ch * 512:(ch + 1) * 512]),
                                         start=True, stop=True)
                        nc.scalar.activation(scr, cps, AF.Square,
                                             accum_out=sqB[:, t * 5 + ch:t * 5 + ch + 1])
                        mx = wp.tile([64, P], F32, tag="mx")
                        nc.vector.tensor_reduce(
                            out=mx, in_=cps[:].rearrange("c (s p) -> c p s", s=4),
                            op=ALU.max, axis=AX.X)
                        if ch == 0:
                            nc.vector.tensor_copy(maxY[:, t * P:(t + 1) * P], mx)
                        else:
                            nc.vector.tensor_tensor(maxY[:, t * P:(t + 1) * P],
                                                    maxY[:, t * P:(t + 1) * P], mx, op=ALU.max)
                # stats: sum_y = W_b @ sum_e1 ; sumsq from sqB
                st = pool.tile([64, 2], F32, tag=f"st_b{ec}")
                se = pool.tile([64, 1], F32, tag="se")
                nc.vector.tensor_reduce(out=se, in_=accS[:], op=ALU.add, axis=AX.X)
                sp = pp.tile([64, 1], F32, tag="sp")
                nc.tensor.matmul(sp, lhsT=wbT, rhs=se[:], start=True, stop=True)
                nc.scalar.copy(st[:, 0:1], sp)
                nc.vector.tensor_reduce(out=st[:, 1:2], in_=sqB[:], op=ALU.add, axis=AX.X)
            allreduce_stats(st[:], 64, 2)
            sc_b, sh_b = bn_coeffs(pool, st[:], gb_b, CNT_E, 64)
            emit_lrelu(pool, x_dst, maxY[:], sc_b, sh_b, tag="lrX")

        # ================= EC1 =================
        with tc.tile_pool(name="ec", bufs=1) as ecp:
            bvT = ecp.tile([P, NT * 64], F32, tag="bvT")
            augL, augR = build_aug(ecp, x_in, 6, from_dram=True)
            wd1 = ecp.tile([6, 64], F32, tag="wd1")
            wcd1 = ecp.tile([6, 64], F32, tag="wcd1")
            wb1 = ecp.tile([64, 64], F32, tag="wb1")
            nc.sync.dma_start(out=wd1, in_=ins["wd1T"])
            nc.sync.dma_start(out=wcd1, in_=ins["wcd1T"])
            nc.sync.dma_start(out=wb1, in_=ins["wb1T"])
            gb1a = ecp.tile([64, 2], F32, tag="gb1a")
            gb1b = ecp.tile([64, 2], F32, tag="gb1b")
            nc.sync.dma_start(out=gb1a, in_=ins["gb1a"])
            nc.sync.dma_start(out=gb1b, in_=ins["gb1b"])
            build_tables(ecp, augR[:], 6, wd1[:], wcd1[:], at_d[0], bvT)
            sc, sh = edgeconv_passA(0, ecp, augL, augR, 6, at_d[0], stg_d[0], gb1a[:],
                                    enc_s4096=float(32 * 4096))
            edgeconv_passB(0, ecp, stg_d[0], wb1[:], sc, sh, gb1b[:], x12[0:64, :])
            if dbg is not None:
                nc.sync.dma_start(out=dbg["x1"], in_=x12[0:64, :])

            # ================= EC2 =================
            wd2 = ecp.tile([64, 64], F32, tag="wd2")
            wcd2 = ecp.tile([64, 64], F32, tag="wcd2")
            wb2 = ecp.tile([64, 64], F32, tag="wb2")
            nc.sync.dma_start(out=wd2, in_=ins["wd2T"])
            nc.sync.dma_start(out=wcd2, in_=ins["wcd2T"])
            nc.sync.dma_start(out=wb2, in_=ins["wb2T"])
            gb2a = ecp.tile([64, 2], F32, tag="gb2a")
            gb2b = ecp.tile([64, 2], F32, tag="gb2b")
            nc.sync.dma_start(out=gb2a, in_=ins["gb2a"])
            nc.sync.dma_start(out=gb2b, in_=ins["gb2b"])
            augL2, augR2 = build_aug(ecp, x12[0:64, :], 64)
            build_tables(ecp, augR2[:], 64, wd2[:], wcd2[:], at_d[1], bvT)
            sc, sh = edgeconv_passA(1, ecp, augL2, augR2, 64, at_d[1], stg_d[1], gb2a[:],
                                    enc_s4096=float(2 * 4096))
            edgeconv_passB(1, ecp, stg_d[1], wb2[:], sc, sh, gb2b[:], x2t[:])
            nc.sync.dma_start(out=x12[64:128, :], in_=x2t[:])
            if dbg is not None:
                nc.sync.dma_start(out=dbg["x2"], in_=x2t[:])

            # ================= EC3 =================
            wd3 = ecp.tile([64, 64], F32, tag="wd3")
            wcd3 = ecp.tile([64, 64], F32, tag="wcd3")
            nc.sync.dma_start(out=wd3, in_=ins["wd3T"])
            nc.sync.dma_start(out=wcd3, in_=ins["wcd3T"])
            gb3 = ecp.tile([64, 2], F32, tag="gb3")
            nc.sync.dma_start(out=gb3, in_=ins["gb3"])
            augL3, augR3 = build_aug(ecp, x2t[:], 64)
            build_tables(ecp, augR3[:], 64, wd3[:], wcd3[:], at_d[2], bvT)
            sc, sh = edgeconv_passA(2, ecp, augL3, augR3, 64, at_d[2], None, gb3[:],
                                    enc_s4096=float(2 * 4096))
            emit_lrelu(ecp, x3t[:], maxE3[:], sc, sh, tag="lrX3")
            if dbg is not None:
                nc.sync.dma_start(out=dbg["x3"], in_=x3t[:])

        # ================= head =================
        with tc.tile_pool(name="head", bufs=1) as hp, \
             tc.tile_pool(name="headp", bufs=2, space="PSUM") as hpp:
            w4a = hp.tile([P, 1024], F32, tag="w4a")
            w4b = hp.tile([64, 1024], F32, tag="w4b")
            nc.sync.dma_start(out=w4a, in_=ins["w4Ta"])
            nc.sync.dma_start(out=w4b, in_=ins["w4Tb"])
            gb4 = hp.tile([P, 16], F32, tag="gb4")
            nc.sync.dma_start(out=gb4, in_=ins["gb4"])

            # conv4: stats + max over N, never materialize h4
            s123a = hp.tile([P, 1], F32, tag="s123a")
            s123b = hp.tile([64, 1], F32, tag="s123b")
            nc.vector.tensor_reduce(out=s123a, in_=x12[:], op=ALU.add, axis=AX.X)
            nc.vector.tensor_reduce(out=s123b, in_=x3t[:], op=ALU.add, axis=AX.X)
            st4 = hp.tile([P, 16], F32, tag="st4")      # (sum, sq) x 8 groups
            mx4 = hp.tile([P, 8], F32, tag="mx4")
            scr4 = hp.tile([P, 512], F32, tag="scr4")
            sq4acc = hp.tile([P, 8 * NC512], F32, tag="sq4acc")
            for g in range(8):
                sp4 = hpp.tile([P, 1], F32, tag="sp")
                nc.tensor.matmul(sp4, lhsT=w4a[:, g * P:(g + 1) * P], rhs=s123a[:],
                                 start=True, stop=False)
                nc.tensor.matmul(sp4, lhsT=w4b[:, g * P:(g + 1) * P], rhs=s123b[:],
                                 start=False, stop=True)
                nc.scalar.copy(st4[:, 2 * g:2 * g + 1], sp4)
                for ch in range(NC512):
                    hps = hpp.tile([P, 512], F32, tag="hps")
                    nc.tensor.matmul(hps, lhsT=r32(w4a[:, g * P:(g + 1) * P]),
                                     rhs=r32(x12[:, ch * 512:(ch + 1) * 512]),
                                     start=True, stop=False)
                    nc.tensor.matmul(hps, lhsT=r32(w4b[:, g * P:(g + 1) * P]),
                                     rhs=r32(x3t[:, ch * 512:(ch + 1) * 512]),
                                     start=False, stop=True)
                    nc.scalar.activation(scr4, hps, AF.Square,
                                         accum_out=sq4acc[:, g * NC512 + ch:g * NC512 + ch + 1])
                    mxc = hp.tile([P, 1], F32, tag="mxc")
                    nc.vector.tensor_reduce(out=mxc, in_=hps[:], op=ALU.max, axis=AX.X)
                    if ch == 0:
                        nc.vector.tensor_copy(mx4[:, g:g + 1], mxc)
                    else:
                        nc.vector.tensor_tensor(mx4[:, g:g + 1], mx4[:, g:g + 1], mxc, op=ALU.max)
            for g in range(8):
                nc.vector.tensor_reduce(out=st4[:, 2 * g + 1:2 * g + 2],
                                        in_=sq4acc[:, g * NC512:(g + 1) * NC512],
                                        op=ALU.add, axis=AX.X)
            allreduce_stats(st4[:], P, 16)
            sc4, sh4 = bn_coeffs(hp, st4[:], gb4[:], CNT_N, P, ncols=8)
            # g4 = lrelu(bn4(max)) : elementwise on [P, 8]
            g4 = hp.tile([P, 8], F32, tag="g4")
            nc.vector.tensor_tensor(g4, mx4, sc4, op=ALU.mult)
            nc.vector.tensor_add(g4, g4, sh4)
            g4n = hp.tile([P, 8], F32, tag="g4n")
            nc.vector.tensor_scalar_mul(g4n, g4, 0.2)
            nc.vector.tensor_tensor(g4, g4, g4n, op=ALU.max)

            # conv5: y5 = W5x @ x123 + (W5g @ g4)
            w5xa = hp.tile([P, 256], F32, tag="w5xa")
            w5xb = hp.tile([64, 256], F32, tag="w5xb")
            nc.sync.dma_start(out=w5xa, in_=ins["w5xTa"])
            nc.sync.dma_start(out=w5xb, in_=ins["w5xTb"])
            gb5 = hp.tile([P, 4], F32, tag="gb5")
            nc.sync.dma_start(out=gb5, in_=ins["gb5"])
            c5 = hp.tile([P, 2], F32, tag="c5")
            with tc.tile_pool(name="w5g", bufs=2) as w5p:
                for og in range(2):
                    c5p = hpp.tile([P, 1], F32, tag="sp")
                    for kc in range(8):
                        w5g = w5p.tile([P, P], F32, tag="w5g")
                        nc.sync.dma_start(out=w5g, in_=ins["w5gT"][kc * P:(kc + 1) * P,
                                                                   og * P:(og + 1) * P])
                        nc.tensor.matmul(c5p, lhsT=w5g[:], rhs=g4[:, kc:kc + 1],
                                         start=(kc == 0), stop=(kc == 7))
                    nc.scalar.copy(c5[:, og:og + 1], c5p)

            h5 = hp.tile([P, 2 * N], F32, tag="h5")
            sq5acc = hp.tile([P, 2 * NC512], F32, tag="sq5acc")
            st5 = hp.tile([P, 4], F32, tag="st5")
            scr5 = hp.tile([P, 512], F32, tag="scr5")
            for og in range(2):
                # sum: W5x @ s123 + N * c5
                sp5 = hpp.tile([P, 1], F32, tag="sp")
                nc.tensor.matmul(sp5, lhsT=w5xa[:, og * P:(og + 1) * P], rhs=s123a[:],
                                 start=True, stop=False)
                nc.tensor.matmul(sp5, lhsT=w5xb[:, og * P:(og + 1) * P], rhs=s123b[:],
                                 start=False, stop=True)
                sc_t = hp.tile([P, 1], F32, tag="sc_t")
                nc.vector.tensor_scalar_mul(sc_t, c5[:, og:og + 1], float(N))
                nc.vector.tensor_copy(st5[:, 2 * og:2 * og + 1], sp5)
                nc.vector.tensor_add(st5[:, 2 * og:2 * og + 1],
                                     st5[:, 2 * og:2 * og + 1], sc_t)
                for ch in range(NC512):
                    hps = hpp.tile([P, 512], F32, tag="hps")
                    nc.tensor.matmul(hps, lhsT=r32(w5xa[:, og * P:(og + 1) * P]),
                                     rhs=r32(x12[:, ch * 512:(ch + 1) * 512]),
                                     start=True, stop=False)
                    nc.tensor.matmul(hps, lhsT=r32(w5xb[:, og * P:(og + 1) * P]),
                                     rhs=r32(x3t[:, ch * 512:(ch + 1) * 512]),
                                     start=False, stop=True)
                    dst = h5[:, og * N + ch * 512: og * N + (ch + 1) * 512]
                    nc.vector.tensor_scalar(dst, hps, c5[:, og:og + 1], None, op0=ALU.add)
                    nc.scalar.activation(scr5, dst, AF.Square,
                                         accum_out=sq5acc[:, og * NC512 + ch:og * NC512 + ch + 1])
            for og in range(2):
                nc.vector.tensor_reduce(out=st5[:, 2 * og + 1:2 * og + 2],
                                        in_=sq5acc[:, og * NC512:(og + 1) * NC512],
                                        op=ALU.add, axis=AX.X)
            allreduce_stats(st5[:], P, 4)
            sc5, sh5 = bn_coeffs(hp, st5[:], gb5[:], CNT_N, P, ncols=2)
            for og in range(2):
                emit_lrelu(hp, h5[:, og * N:(og + 1) * N], h5[:, og * N:(og + 1) * N],
                           sc5[:, og:og + 1], sh5[:, og:og + 1], tag="lrH")

            # conv6: 256 -> 256
            w6 = hp.tile([P, 2 * 256], F32, tag="w6")     # [256,256].T split: rows kc
            nc.sync.dma_start(out=w6, in_=ins["w6T"])     # host packs [128, 512]
            gb6 = hp.tile([P, 4], F32, tag="gb6")
            nc.sync.dma_start(out=gb6, in_=ins["gb6"])
            h6 = hp.tile([P, 2 * N], F32, tag="h6")
            sq6acc = hp.tile([P, 2 * NC512], F32, tag="sq6acc")
            st6 = hp.tile([P, 4], F32, tag="st6")
            s5 = hp.tile([P, 2], F32, tag="s5")
            for og in range(2):
                nc.vector.tensor_reduce(out=s5[:, og:og + 1], in_=h5[:, og * N:(og + 1) * N],
                                        op=ALU.add, axis=AX.X)
            for og in range(2):
                sp6 = hpp.tile([P, 1], F32, tag="sp")
                for kc in range(2):
                    nc.tensor.matmul(sp6, lhsT=w6[:, kc * 256 + og * P: kc * 256 + (og + 1) * P],
                                     rhs=s5[:, kc:kc + 1], start=(kc == 0), stop=(kc == 1))
                nc.scalar.copy(st6[:, 2 * og:2 * og + 1], sp6)
                for ch in range(NC512):
                    hps = hpp.tile([P, 512], F32, tag="hps")
                    for kc in range(2):
                        nc.tensor.matmul(hps,
                                         lhsT=r32(w6[:, kc * 256 + og * P: kc * 256 + (og + 1) * P]),
                                         rhs=r32(h5[:, kc * N + ch * 512: kc * N + (ch + 1) * 512]),
                                         start=(kc == 0), stop=(kc == 1))
                    dst = h6[:, og * N + ch * 512: og * N + (ch + 1) * 512]
                    nc.scalar.activation(scr5, hps, AF.Square,
                                         accum_out=sq6acc[:, og * NC512 + ch:og * NC512 + ch + 1])
                    nc.vector.tensor_copy(dst, hps)
            for og in range(2):
                nc.vector.tensor_reduce(out=st6[:, 2 * og + 1:2 * og + 2],
                                        in_=sq6acc[:, og * NC512:(og + 1) * NC512],
                                        op=ALU.add, axis=AX.X)
            allreduce_stats(st6[:], P, 4)
            sc6, sh6 = bn_coeffs(hp, st6[:], gb6[:], CNT_N, P, ncols=2)
            for og in range(2):
                emit_lrelu(hp, h6[:, og * N:(og + 1) * N], h6[:, og * N:(og + 1) * N],
                           sc6[:, og:og + 1], sh6[:, og:og + 1], tag="lrH")

            # conv7: 256 -> 128
            w7 = hp.tile([P, 2 * P], F32, tag="w7")       # [256,128].T: two [128,128]
            nc.sync.dma_start(out=w7, in_=ins["w7T"])
            gb7 = hp.tile([P, 2], F32, tag="gb7")
            nc.sync.dma_start(out=gb7, in_=ins["gb7"])
            h7 = hp.tile([P, N], F32, tag="h7")
            sq7acc = hp.tile([P, NC512], F32, tag="sq7acc")
            st7 = hp.tile([P, 2], F32, tag="st7")
            s6 = hp.tile([P, 2], F32, tag="s6")
            for og in range(2):
                nc.vector.tensor_reduce(out=s6[:, og:og + 1], in_=h6[:, og * N:(og + 1) * N],
                                        op=ALU.add, axis=AX.X)
            sp7 = hpp.tile([P, 1], F32, tag="sp")
            for kc in range(2):
                nc.tensor.matmul(sp7, lhsT=w7[:, kc * P:(kc + 1) * P], rhs=s6[:, kc:kc + 1],
                                 start=(kc == 0), stop=(kc == 1))
            nc.scalar.copy(st7[:, 0:1], sp7)
            for ch in range(NC512):
                hps = hpp.tile([P, 512], F32, tag="hps")
                for kc in range(2):
                    nc.tensor.matmul(hps, lhsT=r32(w7[:, kc * P:(kc + 1) * P]),
                                     rhs=r32(h6[:, kc * N + ch * 512: kc * N + (ch + 1) * 512]),
                                     start=(kc == 0), stop=(kc == 1))
                dst = h7[:, ch * 512:(ch + 1) * 512]
                nc.scalar.activation(scr5, hps, AF.Square,
                                     accum_out=sq7acc[:, ch:ch + 1])
                nc.vector.tensor_copy(dst, hps)
            nc.vector.tensor_reduce(out=st7[:, 1:2], in_=sq7acc[:], op=ALU.add, axis=AX.X)
            allreduce_stats(st7[:], P, 2)
            sc7, sh7 = bn_coeffs(hp, st7[:], gb7[:], CNT_N, P, ncols=1)
            emit_lrelu(hp, h7[:], h7[:], sc7, sh7, tag="lrH")

            # conv8: 128 -> 6, no bn/act
            w8 = hp.tile([P, 6], F32, tag="w8")
            nc.sync.dma_start(out=w8, in_=ins["w8T"])
            osb = hp.tile([6, N], F32, tag="osb")
            for ch in range(NC512):
                ops = hpp.tile([6, 512], F32, tag="hps")
                nc.tensor.matmul(ops, lhsT=r32(w8[:]), rhs=r32(h7[:, ch * 512:(ch + 1) * 512]),
                                 start=True, stop=True)
                nc.scalar.copy(osb[:, ch * 512:(ch + 1) * 512], ops)
            nc.sync.dma_start(out=out_dram, in_=osb)
        es.close()
    assert n_ar == 9, n_ar


def prep_inputs(inputs, N):
    """Host-side weight/shape prep. Returns per-core in_maps (core c gets
    cloud c) given the full input dict from setup_inputs()."""
    f = {k: np.asarray(v, dtype=np.float32) for k, v in inputs.items()}
    x = f["x"]                            # [8, 6, N]
    B = x.shape[0]

    def gbpair(g, b):
        out = np.stack([g, b], axis=1).astype(np.float32)   # [ch, 2]
        return out

    def gbgrp(g, b, ngr):
        # [ngr*128] channels -> [128, 2*ngr] (g,b) interleaved per group
        out = np.zeros((P, 2 * ngr), np.float32)
        for gi in range(ngr):
            out[:, 2 * gi] = g[gi * P:(gi + 1) * P]
            out[:, 2 * gi + 1] = b[gi * P:(gi + 1) * P]
        return out

    w1a, w1b = f["w1a"], f["w1b"]
    w2a, w2b = f["w2a"], f["w2b"]
    w3, w4, w5, w6, w7, w8 = f["w3"], f["w4"], f["w5"], f["w6"], f["w7"], f["w8"]
    shared = {
        "wd1T": np.ascontiguousarray(w1a[:, :6].T),
        "wcd1T": np.ascontiguousarray((w1a[:, 6:] - w1a[:, :6]).T),
        "wb1T": np.ascontiguousarray(w1b.T),
        "wd2T": np.ascontiguousarray(w2a[:, :64].T),
        "wcd2T": np.ascontiguousarray((w2a[:, 64:] - w2a[:, :64]).T),
        "wb2T": np.ascontiguousarray(w2b.T),
        "wd3T": np.ascontiguousarray(w3[:, :64].T),
        "wcd3T": np.ascontiguousarray((w3[:, 64:] - w3[:, :64]).T),
        "gb1a": gbpair(f["g1a"], f["b1a"]),
        "gb1b": gbpair(f["g1b"], f["b1b"]),
        "gb2a": gbpair(f["g2a"], f["b2a"]),
        "gb2b": gbpair(f["g2b"], f["b2b"]),
        "gb3": gbpair(f["g3"], f["b3"]),
        "w4Ta": np.ascontiguousarray(w4.T[:128, :]),
        "w4Tb": np.ascontiguousarray(w4.T[128:, :]),
        "gb4": gbgrp(f["g4"], f["b4"], 8),
        "w5xTa": np.ascontiguousarray(w5[:, 1024:].T[:128, :]),
        "w5xTb": np.ascontiguousarray(w5[:, 1024:].T[128:, :]),
        "w5gT": np.ascontiguousarray(w5[:, :1024].T),
        "gb5": gbgrp(f["g5"], f["b5"], 2),
        # w6T packed [128, 2*256]: kc-th K-chunk of w6.T at cols kc*256
        "w6T": np.concatenate([w6.T[:128, :], w6.T[128:, :]], axis=1),
        "gb6": gbgrp(f["g6"], f["b6"], 2),
        "w7T": np.concatenate([w7.T[:128, :], w7.T[128:, :]], axis=1),
        "gb7": gbgrp(f["g7"], f["b7"], 1),
        "w8T": np.ascontiguousarray(w8.T),
    }
    shared = {k: np.ascontiguousarray(v, dtype=np.float32) for k, v in shared.items()}
    in_maps = []
    for c in range(B):
        m = dict(shared)
        m["x"] = np.ascontiguousarray(x[c])
        in_maps.append(m)
    return in_maps


_CACHED = {}


def kernel(**inputs) -> np.ndarray:
    from concourse.bass_utils import run_bass_kernel_spmd
    N = int(np.asarray(inputs["x"]).shape[2])
    n_cores = 8
    in_maps = prep_inputs(inputs, N)
    key = (N, n_cores)
    if key not in _CACHED:
        nc = bacc.Bacc("TRN2", target_bir_lowering=False, debug=False,
                       num_devices=n_cores)
        ins = {}
        for k, v in in_maps[0].items():
            ins[k] = nc.dram_tensor(k, list(v.shape), F32, kind="ExternalInput").ap()
        outs = {"out": nc.dram_tensor("out", [6, N], F32, kind="ExternalOutput").ap()}
        build_graph(nc, ins, outs, N, n_cores, lrelu_native=False)
        nc.compile()
        _CACHED[key] = nc
    nc = _CACHED[key]
    res = run_bass_kernel_spmd(nc, in_maps, core_ids=list(range(n_cores)))
    out = np.stack([res.results[c]["out"] for c in range(n_cores)], axis=0)
    return out.astype(np.float32)


def kernel_traced(**inputs):
    """Like kernel() but captures the NTFF profile; returns (out, exec_ns)."""
    from concourse.bass_utils import run_bass_kernel_spmd
    N = int(np.asarray(inputs["x"]).shape[2])
    n_cores = 8
    in_maps = prep_inputs(inputs, N)
    key = (N, n_cores)
    if key not in _CACHED:
        kernel(**inputs)
    nc = _CACHED[key]
    res = run_bass_kernel_spmd(nc, in_maps, core_ids=list(range(n_cores)),
                               trace=True)
    out = np.stack([res.results[c]["out"] for c in range(n_cores)], axis=0)
    return out.astype(np.float32), res.exec_time_ns

